# revision 1
# baseline (speedup 1.0000x reference)
"""Trainium2 Bass kernel for nn_ChunkwiseMLSTM (B=2, S=8192, D=512, INNER=1024, NH=8).

kernel(**inputs) -> np.ndarray [2, 8192, 512] f32.

Three SPMD launches on 8 NeuronCores:
  A: token-sharded projections (up-proj, causal conv, silu, q/k/v, gate pre-acts)
  B: head-sharded chunkwise mLSTM (L=128 chunks; f32 state, bf16 matmuls)
  C: token-sharded output gating + down-projection
Host between launches: gate-derived scan scalars / decay matrices (O(B*NH*S) work),
weight pre-transposition, and resharding.
"""
import os
os.environ.setdefault("JAX_COMPILATION_CACHE_DIR",
                      os.path.expanduser("~/.cache/jax_bass_cache"))
os.environ.setdefault("JAX_PERSISTENT_CACHE_MIN_ENTRY_SIZE_BYTES", "0")
os.environ.setdefault("JAX_PERSISTENT_CACHE_MIN_COMPILE_TIME_SECS", "0")

import sys
if '/opt/trn_rl_repo' not in sys.path:
    sys.path.insert(0, '/opt/trn_rl_repo')

import numpy as np
import ml_dtypes

import concourse.bass as bass
import concourse.tile as tile
from concourse import mybir, bacc

F32 = mybir.dt.float32
BF16 = mybir.dt.bfloat16
AF = mybir.ActivationFunctionType
OP = mybir.AluOpType

B, S, D = 2, 8192, 512
INNER, NH, KCONV = 1024, 8, 4
DH = 128
EPS = 1e-6
LC = 128           # chunk length used on device (math is chunk-size invariant)
NCH = S // LC      # 64
QK_SCALE = DH ** -0.5
TOK = S // 4       # tokens per core in phases A/C = 2048
TH = TOK + (KCONV - 1)   # 2051 with conv halo
NUNIT = 2          # (b,h) units per core in phase B


def _bf(x):
    return np.ascontiguousarray(np.asarray(x, np.float32).astype(ml_dtypes.bfloat16))


def new_nc():
    return bacc.Bacc(None, target_bir_lowering=False, debug=False)


# ---------------------------------------------------------------- phase A ----
def build_phase_a():
    nc = new_nc()
    xt = nc.dram_tensor("xt", [D, TH], BF16, kind="ExternalInput")            # x[b].T slice (halo)
    wupT = nc.dram_tensor("wupT", [D, 2 * INNER], BF16, kind="ExternalInput")  # W_up.T
    wqkvT = nc.dram_tensor("wqkvT", [INNER, 3 * INNER], BF16, kind="ExternalInput")
    wgT = nc.dram_tensor("wgT", [INNER, 2 * NH], BF16, kind="ExternalInput")   # [Wig.T | Wfg.T]
    # host-built diagonal conv weight tiles: diag[p, ft, t, col] = conv_w[ft*128+p, t] * (col==p)
    diag_i = nc.dram_tensor("diag_i", [DH, (INNER // DH) * KCONV * DH], BF16, kind="ExternalInput")
    convb = nc.dram_tensor("convb", [DH, INNER // DH], F32, kind="ExternalInput")

    q_o = nc.dram_tensor("q_o", [INNER, TOK], BF16, kind="ExternalOutput")
    k_o = nc.dram_tensor("k_o", [INNER, TOK], BF16, kind="ExternalOutput")
    v_o = nc.dram_tensor("v_o", [INNER, TOK], BF16, kind="ExternalOutput")
    xm_o = nc.dram_tensor("xm_o", [INNER, TOK], BF16, kind="ExternalOutput")
    xog_o = nc.dram_tensor("xog_o", [INNER, TOK], BF16, kind="ExternalOutput")
    gates_o = nc.dram_tensor("gates_o", [2 * NH, TOK], F32, kind="ExternalOutput")

    KT_UP = D // DH          # 4 k-tiles for up-proj
    MT_UP = 2 * INNER // DH  # 16 m-tiles
    FT = INNER // DH         # 8 feature tiles of the mlstm half
    KT_IN = INNER // DH      # 8 k-tiles over INNER
    MT_QKV = 3 * INNER // DH # 24
    # token n-tiles over TH (matmul N <= 512)
    N_SL = [(i * 512, min(512, TH - i * 512)) for i in range((TH + 511) // 512)]
    N_SL_OUT = [(i * 512, 512) for i in range(TOK // 512)]

    with tile.TileContext(nc) as tc, \
         tc.tile_pool(name="const", bufs=1) as const, \
         tc.tile_pool(name="big", bufs=1) as big, \
         tc.tile_pool(name="wpool", bufs=3) as wpool, \
         tc.tile_pool(name="ev", bufs=4) as ev, \
         tc.tile_pool(name="ps", bufs=4, space="PSUM") as ps:
        if True:
            # --- load x and W_up
            xt_sb = big.tile([DH, KT_UP, TH], BF16)
            nc.sync.dma_start(out=xt_sb, in_=xt[:].rearrange("(kt p) t -> p kt t", p=DH))
            wup_sb = big.tile([DH, KT_UP, 2 * INNER], BF16)
            nc.sync.dma_start(out=wup_sb, in_=wupT[:].rearrange("(kt p) m -> p kt m", p=DH))

            # --- up-projection: x_pre (mlstm half, bf16, kept) and x_og (exported)
            xpre_sb = big.tile([DH, FT, TH], BF16)
            xog_sb = big.tile([DH, FT, TOK], BF16)
            for m in range(MT_UP):
                mlstm_half = m < FT
                for (n0, nn) in (N_SL if mlstm_half else N_SL_OUT):
                    pt = ps.tile([DH, 512], F32)
                    off = 0 if mlstm_half else KCONV - 1
                    for kt in range(KT_UP):
                        nc.tensor.matmul(
                            pt[:, :nn],
                            wup_sb[:, kt, m * DH:(m + 1) * DH],
                            xt_sb[:, kt, off + n0: off + n0 + nn],
                            start=(kt == 0), stop=(kt == KT_UP - 1))
                    if mlstm_half:
                        nc.scalar.copy(xpre_sb[:, m, n0:n0 + nn], pt[:, :nn])
                    else:
                        nc.vector.tensor_copy(xog_sb[:, m - FT, n0:n0 + nn], pt[:, :nn])
            nc.sync.dma_start(out=xog_o[:].rearrange("(ft p) t -> p ft t", p=DH), in_=xog_sb)

            # --- causal depthwise conv as 4 diag matmuls + SiLU
            convb_sb = const.tile([DH, FT], F32)
            nc.sync.dma_start(out=convb_sb, in_=convb[:])
            diag = const.tile([DH, FT, KCONV, DH], BF16)
            nc.sync.dma_start(
                out=diag, in_=diag_i[:].rearrange("p (ft t c) -> p ft t c", ft=FT, t=KCONV))

            xm_sb = big.tile([DH, FT, TOK], BF16)
            for ft in range(FT):
                for (n0, nn) in N_SL_OUT:
                    pt = ps.tile([DH, 512], F32)
                    for t in range(KCONV):
                        nc.tensor.matmul(
                            pt[:, :nn],
                            diag[:, ft, t, :],
                            xpre_sb[:, ft, n0 + t: n0 + t + nn],
                            start=(t == 0), stop=(t == KCONV - 1))
                    # silu(y+b) = (y+b) * sigmoid(y+b)   (Silu LUT unavailable in CoreSim)
                    sg_t = ev.tile([DH, 512], BF16, tag="sg")
                    nc.scalar.activation(
                        sg_t[:, :nn], pt[:, :nn], AF.Sigmoid,
                        bias=convb_sb[:, ft:ft + 1], scale=1.0)
                    nc.vector.scalar_tensor_tensor(
                        xm_sb[:, ft, n0:n0 + nn], pt[:, :nn], convb_sb[:, ft:ft + 1],
                        sg_t[:, :nn], OP.add, OP.mult)
            nc.sync.dma_start(out=xm_o[:].rearrange("(ft p) t -> p ft t", p=DH), in_=xm_sb)

            # --- q/k/v projections (streamed weights) + gates
            qkv_outs = [q_o, k_o, v_o]
            for m in range(MT_QKV):
                w_sb = wpool.tile([DH, KT_IN, DH], BF16)
                nc.sync.dma_start(
                    out=w_sb,
                    in_=wqkvT[:, m * DH:(m + 1) * DH].rearrange("(kt p) m -> p kt m", p=DH))
                out_t = qkv_outs[m // FT]
                mf = m % FT
                for (n0, nn) in N_SL_OUT:
                    pt = ps.tile([DH, 512], F32)
                    for kt in range(KT_IN):
                        nc.tensor.matmul(
                            pt[:, :nn], w_sb[:, kt, :], xm_sb[:, kt, n0:n0 + nn],
                            start=(kt == 0), stop=(kt == KT_IN - 1))
                    ev_t = ev.tile([DH, 512], BF16)
                    if m % 2 == 0:
                        nc.scalar.copy(ev_t[:, :nn], pt[:, :nn])
                    else:
                        nc.vector.tensor_copy(ev_t[:, :nn], pt[:, :nn])
                    nc.sync.dma_start(
                        out=out_t[mf * DH:(mf + 1) * DH, n0:n0 + nn], in_=ev_t[:, :nn])

            # gates: [16, TOK]
            wg_sb = const.tile([DH, KT_IN, 2 * NH], BF16)
            nc.sync.dma_start(out=wg_sb, in_=wgT[:].rearrange("(kt p) m -> p kt m", p=DH))
            for (n0, nn) in N_SL_OUT:
                pt = ps.tile([2 * NH, 512], F32)
                for kt in range(KT_IN):
                    nc.tensor.matmul(
                        pt[:, :nn], wg_sb[:, kt, :], xm_sb[:, kt, n0:n0 + nn],
                        start=(kt == 0), stop=(kt == KT_IN - 1))
                gv = ev.tile([2 * NH, 512], F32)
                nc.vector.tensor_copy(gv[:, :nn], pt[:, :nn])
                nc.sync.dma_start(out=gates_o[:, n0:n0 + nn], in_=gv[:, :nn])
    nc.compile()
    return nc


# ---------------------------------------------------------------- phase B ----
def build_phase_b(interleave=True, cast_engine="gpsimd", ablate=()):
    ablate = set(ablate)
    nc = new_nc()
    NW = 132  # padded width for [C|n] and [v|1]
    ins = {}
    outs = {}
    for u in range(NUNIT):
        ins[f"qT{u}"] = nc.dram_tensor(f"qT{u}", [DH, S], BF16, kind="ExternalInput")
        ins[f"kT{u}"] = nc.dram_tensor(f"kT{u}", [DH, S], BF16, kind="ExternalInput")
        ins[f"kesc{u}"] = nc.dram_tensor(f"kesc{u}", [S, DH], BF16, kind="ExternalInput")
        ins[f"vone{u}"] = nc.dram_tensor(f"vone{u}", [S, NW], BF16, kind="ExternalInput")
        ins[f"DpT{u}"] = nc.dram_tensor(f"DpT{u}", [S, DH], BF16, kind="ExternalInput")
        ins[f"dec{u}"] = nc.dram_tensor(f"dec{u}", [DH, NCH], F32, kind="ExternalInput")
        ins[f"e2{u}"] = nc.dram_tensor(f"e2{u}", [DH, NCH], F32, kind="ExternalInput")
        ins[f"e3{u}"] = nc.dram_tensor(f"e3{u}", [DH, NCH], F32, kind="ExternalInput")
        outs[f"h{u}"] = nc.dram_tensor(f"h{u}", [S, DH], F32, kind="ExternalOutput")

    with tile.TileContext(nc) as tc, \
         tc.tile_pool(name="big", bufs=1) as big, \
         tc.tile_pool(name="work", bufs=4) as work, \
         tc.tile_pool(name="hpool", bufs=4) as hpool, \
         tc.tile_pool(name="state", bufs=1) as state, \
         tc.tile_pool(name="ps_s", bufs=2, space="PSUM") as ps_s, \
         tc.tile_pool(name="ps_num", bufs=3, space="PSUM") as ps_num, \
         tc.tile_pool(name="ps_u", bufs=2, space="PSUM") as ps_u:
        if True:
            T = {}
            for u in range(NUNIT):
                T[u] = dict(
                    qT=big.tile([DH, S], BF16, tag=f"qT{u}", name=f"qT{u}"),
                    kT=big.tile([DH, S], BF16, tag=f"kT{u}", name=f"kT{u}"),
                    kesc=big.tile([DH, NCH, DH], BF16, tag=f"kesc{u}", name=f"kesc{u}"),
                    vone=big.tile([DH, NCH, NW], BF16, tag=f"vone{u}", name=f"vone{u}"),
                    DpT=big.tile([DH, NCH, DH], BF16, tag=f"DpT{u}", name=f"DpT{u}"),
                    dec=big.tile([DH, NCH], F32, tag=f"dec{u}", name=f"dec{u}"),
                    e2=big.tile([DH, NCH], F32, tag=f"e2{u}", name=f"e2{u}"),
                    e3=big.tile([DH, NCH], F32, tag=f"e3{u}", name=f"e3{u}"),
                    Cf=state.tile([DH, NW], F32, tag=f"Cf{u}", name=f"Cf{u}"),
                    Cb=state.tile([DH, NW], BF16, tag=f"Cb{u}", name=f"Cb{u}"),
                )
                t = T[u]
                nc.sync.dma_start(out=t['qT'], in_=ins[f"qT{u}"][:])
                nc.sync.dma_start(out=t['kT'], in_=ins[f"kT{u}"][:])
                nc.sync.dma_start(out=t['kesc'], in_=ins[f"kesc{u}"][:].rearrange("(c p) e -> p c e", p=DH))
                nc.sync.dma_start(out=t['vone'], in_=ins[f"vone{u}"][:].rearrange("(c p) e -> p c e", p=DH))
                nc.sync.dma_start(out=t['DpT'], in_=ins[f"DpT{u}"][:].rearrange("(c p) e -> p c e", p=DH))
                nc.sync.dma_start(out=t['dec'], in_=ins[f"dec{u}"][:])
                nc.sync.dma_start(out=t['e2'], in_=ins[f"e2{u}"][:])
                nc.sync.dma_start(out=t['e3'], in_=ins[f"e3{u}"][:])
                nc.vector.memset(t['Cf'][:], 0.0)
                nc.vector.memset(t['Cb'][:], 0.0)

            def chunk_body(u, c):
                t = T[u]
                csl = slice(c * LC, (c + 1) * LC)
                # mm1: S_T[j,l] = k_c @ q_c^T
                s_ps = ps_s.tile([DH, DH], F32, tag="s_ps")
                if 'mm1' not in ablate:
                    nc.tensor.matmul(s_ps[:], t['kT'][:, csl], t['qT'][:, csl], start=True, stop=True)
                # Sp = S_T * Dp^T   (masked, scaled)  -> bf16
                sp = work.tile([DH, DH], BF16, tag="sp")
                if 'sp' not in ablate:
                    nc.vector.tensor_tensor(sp[:], s_ps[:], t['DpT'][:, c, :], OP.mult)
                # num = q_c^T-mm @ [C|n]  +  Sp^T @ [v|1]
                num = ps_num.tile([DH, NW], F32, tag="num")
                if 'num' not in ablate:
                    nc.tensor.matmul(num[:], t['qT'][:, csl], t['Cb'][:], start=True, stop=False)
                    nc.tensor.matmul(num[:], sp[:], t['vone'][:, c, :], start=False, stop=True)
                # den = abs_max(num[:,128], e2) + e3 ; rden = 1/den
                den = work.tile([DH, 1], F32, tag="den")
                absd = work.tile([DH, 1], F32, tag="absd")
                rden = work.tile([DH, 1], F32, tag="rden")
                if 'den' not in ablate:
                    # den = max(|den_raw|, e2) + e3   (abs_max not walrus-legal)
                    nc.scalar.activation(absd[:], num[:, DH:DH + 1], AF.Abs)
                    nc.vector.scalar_tensor_tensor(
                        den[:], absd[:], t['e2'][:, c:c + 1], t['e3'][:, c:c + 1],
                        OP.max, OP.add)
                    nc.vector.reciprocal(rden[:], den[:])
                # h_c = num[:, :128] * rden   (ACT copy with per-partition scale)
                h_sb = hpool.tile([DH, DH], F32, tag="h_sb")
                if 'h' not in ablate:
                    nc.scalar.activation(h_sb[:], num[:, :DH], AF.Copy, bias=0.0, scale=rden[:])
                    nc.sync.dma_start(out=outs[f"h{u}"][csl, :], in_=h_sb[:])
                # mm2: U = kesc_c^T @ [v|1]
                u_ps = ps_u.tile([DH, NW], F32, tag="u_ps")
                if 'mm2' not in ablate:
                    nc.tensor.matmul(u_ps[:], t['kesc'][:, c, :], t['vone'][:, c, :], start=True, stop=True)
                # C = C*dec + U ;  Cb = bf16(C)
                if 'stt' not in ablate:
                    nc.vector.scalar_tensor_tensor(
                        t['Cf'][:], t['Cf'][:], t['dec'][:, c:c + 1], u_ps[:], OP.mult, OP.add)
                if 'cast' not in ablate:
                    if cast_engine == "gpsimd":
                        nc.gpsimd.tensor_copy(t['Cb'][:], t['Cf'][:])
                    else:
                        nc.scalar.copy(t['Cb'][:], t['Cf'][:])

            if interleave:
                for c in range(NCH):
                    for u in range(NUNIT):
                        chunk_body(u, c)
            else:
                for u in range(NUNIT):
                    for c in range(NCH):
                        chunk_body(u, c)
    nc.compile()
    return nc


# ---------------------------------------------------------------- phase C ----
def build_phase_c():
    nc = new_nc()
    h_i = nc.dram_tensor("h_i", [INNER, TOK], BF16, kind="ExternalInput")
    xm_i = nc.dram_tensor("xm_i", [INNER, TOK], BF16, kind="ExternalInput")
    xog_i = nc.dram_tensor("xog_i", [INNER, TOK], BF16, kind="ExternalInput")
    skip_i = nc.dram_tensor("skip_i", [DH, INNER // DH], F32, kind="ExternalInput")
    wdT = nc.dram_tensor("wdT", [INNER, D], BF16, kind="ExternalInput")
    out_o = nc.dram_tensor("out_o", [D, TOK], F32, kind="ExternalOutput")

    FT = INNER // DH   # 8
    MT = D // DH       # 4
    N_SL = [(i * 512, 512) for i in range(TOK // 512)]
    with tile.TileContext(nc) as tc, \
         tc.tile_pool(name="big", bufs=1) as big, \
         tc.tile_pool(name="ev", bufs=4) as ev, \
         tc.tile_pool(name="ps", bufs=4, space="PSUM") as ps:
        if True:
            h_sb = big.tile([DH, FT, TOK], BF16)
            xm_sb = big.tile([DH, FT, TOK], BF16)
            xog_sb = big.tile([DH, FT, TOK], BF16)
            skip_sb = big.tile([DH, FT], F32)
            wd_sb = big.tile([DH, FT, D], BF16)
            nc.sync.dma_start(out=h_sb, in_=h_i[:].rearrange("(ft p) t -> p ft t", p=DH))
            nc.sync.dma_start(out=xm_sb, in_=xm_i[:].rearrange("(ft p) t -> p ft t", p=DH))
            nc.sync.dma_start(out=xog_sb, in_=xog_i[:].rearrange("(ft p) t -> p ft t", p=DH))
            nc.sync.dma_start(out=skip_sb, in_=skip_i[:])
            nc.sync.dma_start(out=wd_sb, in_=wdT[:].rearrange("(ft p) m -> p ft m", p=DH))

            hg_sb = big.tile([DH, FT, TOK], BF16)
            for ft in range(FT):
                g = ev.tile([DH, TOK], BF16, tag="g")
                nc.scalar.activation(g[:], xog_sb[:, ft, :], AF.Sigmoid)
                g2 = ev.tile([DH, TOK], BF16, tag="g2")
                nc.vector.tensor_tensor(g2[:], xog_sb[:, ft, :], g[:], OP.mult)
                hs = ev.tile([DH, TOK], BF16, tag="hs")
                nc.vector.scalar_tensor_tensor(
                    hs[:], xm_sb[:, ft, :], skip_sb[:, ft:ft + 1], h_sb[:, ft, :],
                    OP.mult, OP.add)
                nc.vector.tensor_tensor(hg_sb[:, ft, :], hs[:], g2[:], OP.mult)

            for m in range(MT):
                for (n0, nn) in N_SL:
                    pt = ps.tile([DH, 512], F32)
                    for kt in range(FT):
                        nc.tensor.matmul(
                            pt[:, :nn], wd_sb[:, kt, m * DH:(m + 1) * DH],
                            hg_sb[:, kt, n0:n0 + nn],
                            start=(kt == 0), stop=(kt == FT - 1))
                    ot = ev.tile([DH, 512], F32, tag="ot")
                    nc.vector.tensor_copy(ot[:, :nn], pt[:, :nn])
                    nc.sync.dma_start(out=out_o[m * DH:(m + 1) * DH, n0:n0 + nn], in_=ot[:, :nn])
    nc.compile()
    return nc


# ------------------------------------------------------------- host glue ----
def host_gate_math(i_pre, f_pre):
    """i_pre, f_pre: [B, NH, S] f32.  Returns dict of f32 arrays."""
    i_pre = i_pre.astype(np.float64)
    f_pre = f_pre.astype(np.float64)
    vecI = np.log(1.0 / (1.0 + np.exp(-i_pre)) + EPS)
    vecF = np.log(1.0 / (1.0 + np.exp(-f_pre)) + EPS)
    Ic = vecI.reshape(B, NH, NCH, LC)
    Fc = vecF.reshape(B, NH, NCH, LC)
    vecB = np.cumsum(Fc, axis=-1)
    scaG = vecB[..., -1]
    vecA = scaG[..., None] - vecB + Ic

    ms = np.zeros((B, NH, NCH))
    dec = np.zeros((B, NH, NCH))
    m_new_arr = np.zeros((B, NH, NCH))
    m = np.zeros((B, NH))
    for c in range(NCH):
        amax = vecA[:, :, c, :].max(-1)
        m_new = np.maximum(scaG[:, :, c] + m, amax)
        ms[:, :, c] = m
        dec[:, :, c] = np.exp(scaG[:, :, c] + m - m_new)
        m_new_arr[:, :, c] = m_new
        m = m_new
    escale = np.exp(vecA - m_new_arr[..., None])

    mask = np.tril(np.ones((LC, LC), bool))
    logD = vecB[..., :, None] - vecB[..., None, :] + Ic[..., None, :]
    logD = np.where(mask, logD, -np.inf)
    m_intra = logD.max(-1)
    m_comb = np.maximum(vecB + ms[..., None], m_intra)
    Dp = np.where(mask, np.exp((Ic - vecB)[..., None, :] - ms[..., None, None]), 0.0)
    e2 = np.exp(-vecB - ms[..., None]) / QK_SCALE
    e3 = EPS * np.exp(m_comb - vecB - ms[..., None]) / QK_SCALE
    return dict(
        escale=escale.astype(np.float32), dec=dec.astype(np.float32),
        DpT=np.ascontiguousarray(Dp.transpose(0, 1, 2, 4, 3)).astype(np.float32),
        e2=e2.astype(np.float32), e3=e3.astype(np.float32))


def prep_weights(W_up, Wq, Wk, Wv, W_ig, W_fg, conv_w, conv_b, skip, W_down):
    """Host-side weight packing (same for all cores)."""
    FT = INNER // DH
    wupT = _bf(W_up.T)                                         # [512, 2048]
    wqkvT = _bf(np.concatenate([Wq.T, Wk.T, Wv.T], axis=1))    # [1024, 3072]
    wgT = _bf(np.concatenate([W_ig.T, W_fg.T], axis=1))        # [1024, 16]
    # diag[p, ft, t, col] = conv_w[ft*128+p, t] * (col == p)
    diag = np.zeros((DH, FT, KCONV, DH), np.float32)
    idx = np.arange(DH)
    cw = conv_w.reshape(FT, DH, KCONV)
    for ft in range(FT):
        for t in range(KCONV):
            diag[idx, ft, t, idx] = cw[ft, :, t]
    diag_i = _bf(diag.reshape(DH, FT * KCONV * DH))
    convb = np.ascontiguousarray(conv_b.reshape(FT, DH).T).astype(np.float32)
    skip_p = np.ascontiguousarray(skip.reshape(FT, DH).T).astype(np.float32)
    wdT = _bf(W_down.T)                                        # [1024, 512]
    return dict(wupT=wupT, wqkvT=wqkvT, wgT=wgT, diag_i=diag_i, convb=convb,
                skip_p=skip_p, wdT=wdT)


def build_a_inmaps(x, wp, b_ig, b_fg):
    """Per-core phase A input maps.  Core c = (b=c//4, quarter=c%4)."""
    in_maps = []
    for c in range(8):
        b, qt = c // 4, c % 4
        s0 = qt * TOK
        xs = x[b, :, :].T                                       # [512, S] view
        if s0 == 0:
            xt = np.concatenate([np.zeros((D, KCONV - 1), np.float32),
                                 xs[:, :TOK]], axis=1)
        else:
            xt = xs[:, s0 - (KCONV - 1): s0 + TOK]
        in_maps.append(dict(
            xt=_bf(xt), wupT=wp['wupT'], wqkvT=wp['wqkvT'], wgT=wp['wgT'],
            diag_i=wp['diag_i'], convb=wp['convb']))
    return in_maps


def assemble_a_outputs(a_results, b_ig, b_fg):
    """Concatenate per-core phase A outputs into full feature-major tensors.

    Returns q_t,k_t,v_t,xm_t,xog_t as [B, INNER, S] (bf16-valued f32 arrays
    stay in ml_dtypes.bfloat16) and i_pre,f_pre [B, NH, S] f32 (bias added)."""
    def cat(name):
        return np.stack([
            np.concatenate([a_results[b * 4 + qt][name] for qt in range(4)], axis=1)
            for b in range(B)])
    q_t, k_t, v_t = cat('q_o'), cat('k_o'), cat('v_o')          # [B, INNER, S] bf16
    xm_t, xog_t = cat('xm_o'), cat('xog_o')
    gates = cat('gates_o').astype(np.float32)                   # [B, 16, S]
    i_pre = gates[:, :NH, :] + np.asarray(b_ig, np.float32)[None, :, None]
    f_pre = gates[:, NH:, :] + np.asarray(b_fg, np.float32)[None, :, None]
    return q_t, k_t, v_t, xm_t, xog_t, i_pre, f_pre


def build_b_inmaps(q_t, k_t, v_t, g):
    """Per-core phase B inputs.  Core c handles units (b, 2h) where
    b = c // 4, heads (2*(c%4), 2*(c%4)+1)."""
    NW = 132
    in_maps = []
    for c in range(8):
        b, hp = c // 4, c % 4
        m = {}
        for u in range(NUNIT):
            h = 2 * hp + u
            rs = slice(h * DH, (h + 1) * DH)
            qT = np.ascontiguousarray(q_t[b, rs, :])            # [128, S] bf16
            kT = np.ascontiguousarray(k_t[b, rs, :])
            k_tok = k_t[b, rs, :].T.astype(np.float32)          # [S, 128]
            esc = g['escale'][b, h].reshape(S)                  # [S]
            kesc = _bf(k_tok * esc[:, None])
            vone = np.zeros((S, NW), np.float32)
            vone[:, :DH] = v_t[b, rs, :].T.astype(np.float32)
            vone[:, DH] = 1.0
            DpT = _bf(g['DpT'][b, h].reshape(S, DH))            # [NCH*128(j), 128(l)]
            m[f"qT{u}"] = qT
            m[f"kT{u}"] = kT
            m[f"kesc{u}"] = kesc
            m[f"vone{u}"] = _bf(vone)
            m[f"DpT{u}"] = DpT
            m[f"dec{u}"] = np.ascontiguousarray(
                np.broadcast_to(g['dec'][b, h][None, :], (DH, NCH)).astype(np.float32))
            m[f"e2{u}"] = np.ascontiguousarray(g['e2'][b, h].T.astype(np.float32))
            m[f"e3{u}"] = np.ascontiguousarray(g['e3'][b, h].T.astype(np.float32))
        in_maps.append(m)
    return in_maps


def build_c_inmaps(b_results, xm_t, xog_t, wp):
    """Assemble h from phase B and build per-core phase C inputs."""
    # h per (b, h): [S, 128] f32 -> full feature-major h_t [B, INNER, S] bf16
    h_t = np.empty((B, INNER, S), np.float32)
    for c in range(8):
        b, hp = c // 4, c % 4
        for u in range(NUNIT):
            h = 2 * hp + u
            h_t[b, h * DH:(h + 1) * DH, :] = b_results[c][f"h{u}"].T
    in_maps = []
    for c in range(8):
        b, qt = c // 4, c % 4
        ts = slice(qt * TOK, (qt + 1) * TOK)
        in_maps.append(dict(
            h_i=_bf(h_t[b, :, ts]),
            xm_i=np.ascontiguousarray(xm_t[b, :, ts]),
            xog_i=np.ascontiguousarray(xog_t[b, :, ts]),
            skip_i=wp['skip_p'], wdT=wp['wdT']))
    return in_maps


def assemble_output(c_results):
    out = np.empty((B, S, D), np.float32)
    for c in range(8):
        b, qt = c // 4, c % 4
        out[b, qt * TOK:(qt + 1) * TOK, :] = c_results[c]['out_o'].T
    return out


# ------------------------------------------------------------------ entry ----
from concourse.bass_utils import run_bass_kernel_spmd as _run_spmd

_CACHE = {}


def _programs():
    if 'a' not in _CACHE:
        _CACHE['a'] = build_phase_a()
        _CACHE['b'] = build_phase_b()
        _CACHE['c'] = build_phase_c()
    return _CACHE['a'], _CACHE['b'], _CACHE['c']


def kernel(x, W_up, Wq, Wk, Wv, W_ig, b_ig, W_fg, b_fg, conv_w, conv_b, skip,
           W_down):
    x = np.asarray(x, np.float32)
    nc_a, nc_b, nc_c = _programs()
    cores = list(range(8))
    wp = prep_weights(W_up, Wq, Wk, Wv, W_ig, W_fg, conv_w, conv_b, skip, W_down)
    a_maps = build_a_inmaps(x, wp, b_ig, b_fg)
    ra = _run_spmd(nc_a, a_maps, core_ids=cores).results
    q_t, k_t, v_t, xm_t, xog_t, i_pre, f_pre = assemble_a_outputs(ra, b_ig, b_fg)
    g = host_gate_math(i_pre, f_pre)
    b_maps = build_b_inmaps(q_t, k_t, v_t, g)
    rb = _run_spmd(nc_b, b_maps, core_ids=cores).results
    c_maps = build_c_inmaps(rb, xm_t, xog_t, wp)
    rc = _run_spmd(nc_c, c_maps, core_ids=cores).results
    return assemble_output(rc)



# revision 25
# speedup vs baseline: 1.2088x; 1.2088x over previous
"""Trainium2 Bass kernel for nn_ChunkwiseMLSTM (B=2, S=8192, D=512, INNER=1024, NH=8).

kernel(**inputs) -> np.ndarray [2, 8192, 512] f32.

Three SPMD launches on 8 NeuronCores:
  A: token-sharded projections (up-proj, DVE causal conv+SiLU, q/k/v, gate
     pre-acts, pre-gated skip*xm and silu(x_og))
  B: head-sharded chunkwise mLSTM (LC=128 chunks), batched:
     - stage i: all inter-chunk state updates U_c = kesc_c^T @ [v|1] (PE),
       PSUM->SBUF bf16 copies (ACT), then ONE tensor_tensor_scan per unit
       (Pool) computes every chunk-boundary state [C|n] with f32 carry.
     - stage ii: per 4-chunk blocks: S_mat matmuls (PE), mask-mult (DVE),
       num matmuls against scanned states (PE), batched den (ACT/DVE),
       per-chunk h scale-copy (ACT/DVE alternating).
  C: token-sharded output gating + down-projection, token-halved pipeline.
Host between launches: gate-derived scan scalars (O(B*NH*S)), weight
pre-transposition, resharding.
"""
import os
os.environ.setdefault("JAX_COMPILATION_CACHE_DIR",
                      os.path.expanduser("~/.cache/jax_bass_cache"))
os.environ.setdefault("JAX_PERSISTENT_CACHE_MIN_ENTRY_SIZE_BYTES", "0")
os.environ.setdefault("JAX_PERSISTENT_CACHE_MIN_COMPILE_TIME_SECS", "0")

import sys
if '/opt/trn_rl_repo' not in sys.path:
    sys.path.insert(0, '/opt/trn_rl_repo')

import numpy as np
import ml_dtypes

import concourse.bass as bass
import concourse.tile as tile
from concourse import mybir, bacc

F32 = mybir.dt.float32
BF16 = mybir.dt.bfloat16
AF = mybir.ActivationFunctionType
OP = mybir.AluOpType

B, S, D = 2, 8192, 512
INNER, NH, KCONV = 1024, 8, 4
DH = 128
EPS = 1e-6
LC = 128           # chunk length used on device (math is chunk-size invariant)
NCH = S // LC      # 64
NW = 129           # [C | n] state width
QK_SCALE = DH ** -0.5
TOK = S // 4       # tokens per core in phases A/C = 2048
TH = TOK + (KCONV - 1)   # 2051 with conv halo
NUNIT = 2          # (b,h) units per core in phase B


def _bf(x):
    return np.ascontiguousarray(np.asarray(x, np.float32).astype(ml_dtypes.bfloat16))


def new_nc():
    return bacc.Bacc(None, target_bir_lowering=False, debug=False)


# ---------------------------------------------------------------- phase A ----
def build_phase_a():
    nc = new_nc()
    xt = nc.dram_tensor("xt", [D, TH], BF16, kind="ExternalInput")            # x[b].T slice (halo)
    wupT = nc.dram_tensor("wupT", [D, 2 * INNER], BF16, kind="ExternalInput")  # W_up.T
    wqkvT = nc.dram_tensor("wqkvT", [INNER, 3 * INNER], BF16, kind="ExternalInput")
    wgT = nc.dram_tensor("wgT", [INNER, 2 * NH], BF16, kind="ExternalInput")   # [Wig.T | Wfg.T]
    convw = nc.dram_tensor("convw", [DH, (INNER // DH) * KCONV], F32, kind="ExternalInput")
    convb = nc.dram_tensor("convb", [DH, INNER // DH], F32, kind="ExternalInput")
    skip_i = nc.dram_tensor("skip_i", [DH, INNER // DH], F32, kind="ExternalInput")

    q_o = nc.dram_tensor("q_o", [INNER, TOK], BF16, kind="ExternalOutput")
    k_o = nc.dram_tensor("k_o", [INNER, TOK], BF16, kind="ExternalOutput")
    v_o = nc.dram_tensor("v_o", [INNER, TOK], BF16, kind="ExternalOutput")
    sxm_o = nc.dram_tensor("sxm_o", [INNER, TOK], BF16, kind="ExternalOutput")   # skip * xm
    g2_o = nc.dram_tensor("g2_o", [INNER, TOK], BF16, kind="ExternalOutput")     # silu(x_og)
    gates_o = nc.dram_tensor("gates_o", [2 * NH, TOK], F32, kind="ExternalOutput")

    KT_UP = D // DH          # 4 k-tiles for up-proj
    FT = INNER // DH         # 8 feature tiles of the mlstm half
    MT_UP = 2 * FT           # 16 m-tiles of up-proj
    KT_IN = INNER // DH      # 8 k-tiles over INNER
    MT_QKV = 3 * FT          # 24
    N_SL = [(i * 512, min(512, TH - i * 512)) for i in range((TH + 511) // 512)]
    N_SL_OUT = [(i * 512, 512) for i in range(TOK // 512)]

    with tile.TileContext(nc) as tc, \
         tc.tile_pool(name="const", bufs=1) as const, \
         tc.tile_pool(name="big", bufs=1) as big, \
         tc.tile_pool(name="wpool", bufs=3) as wpool, \
         tc.tile_pool(name="ev", bufs=4) as ev, \
         tc.tile_pool(name="cv", bufs=3) as cv, \
         tc.tile_pool(name="ps", bufs=4, space="PSUM") as ps:
        if True:
            # --- load x and W_up
            wup_sb = big.tile([DH, KT_UP, 2 * INNER], BF16)
            nc.sync.dma_start(out=wup_sb[:, :, :INNER],
                              in_=wupT[:, :INNER].rearrange("(kt p) m -> p kt m", p=DH))
            xt_sb = big.tile([DH, KT_UP, TH], BF16)
            for kt in range(KT_UP):
                nc.sync.dma_start(out=xt_sb[:, kt, :],
                                  in_=xt[kt * DH:(kt + 1) * DH, :])
            nc.sync.dma_start(out=wup_sb[:, :, INNER:],
                              in_=wupT[:, INNER:].rearrange("(kt p) m -> p kt m", p=DH))
            convw_sb = const.tile([DH, FT, KCONV], F32)
            nc.sync.dma_start(out=convw_sb, in_=convw[:].rearrange("p (ft t) -> p ft t", ft=FT))
            convb_sb = const.tile([DH, FT], F32)
            nc.sync.dma_start(out=convb_sb, in_=convb[:])
            skip_sb = const.tile([DH, FT], F32)
            nc.sync.dma_start(out=skip_sb, in_=skip_i[:])

            # --- up-projection mlstm half: x_pre (bf16, with halo, kept)
            xpre_sb = big.tile([DH, FT, TH], BF16)
            xog_sb = big.tile([DH, FT, TOK], BF16)
            for m in range(FT):
                for (n0, nn) in N_SL:
                    pt = ps.tile([DH, 512], F32)
                    for kt in range(KT_UP):
                        nc.tensor.matmul(
                            pt[:, :nn],
                            wup_sb[:, kt, m * DH:(m + 1) * DH],
                            xt_sb[:, kt, n0: n0 + nn],
                            start=(kt == 0), stop=(kt == KT_UP - 1))
                    nc.scalar.copy(xpre_sb[:, m, n0:n0 + nn], pt[:, :nn])

            # --- causal depthwise conv + SiLU, token halves, alternating
            # DVE/Pool per feature tile so the two engines run in parallel.
            # y[f, t] = sum_tau w[f, tau] * xpre[f, t + tau]
            xm_sb = big.tile([DH, FT, TOK], BF16)
            HT = TOK // 2
            for half in range(2):
                h0 = half * HT
                for ft in range(FT):
                    ce = nc.vector
                    xe = nc.vector
                    a = None
                    for tau in range(KCONV):
                        an = cv.tile([DH, HT], BF16, tag="acc")
                        if tau == 0:
                            ce.tensor_scalar_mul(
                                an[:], xpre_sb[:, ft, h0:h0 + HT],
                                convw_sb[:, ft, 0:1])
                        else:
                            ce.scalar_tensor_tensor(
                                an[:], xpre_sb[:, ft, h0 + tau:h0 + tau + HT],
                                convw_sb[:, ft, tau:tau + 1], a[:],
                                OP.mult, OP.add)
                        a = an
                    # silu(y+b) = (y+b) * sigmoid(y+b)
                    sg_t = cv.tile([DH, HT], BF16, tag="sg")
                    nc.scalar.activation(
                        sg_t[:], a[:], AF.Sigmoid,
                        bias=convb_sb[:, ft:ft + 1], scale=1.0)
                    xe.scalar_tensor_tensor(
                        xm_sb[:, ft, h0:h0 + HT], a[:], convb_sb[:, ft:ft + 1],
                        sg_t[:], OP.add, OP.mult)


            # --- skip * xm (DVE slack while PE runs qkv)
            for ft in range(FT):
                sxm_t = cv.tile([DH, TOK], BF16, tag="out")
                nc.vector.tensor_scalar_mul(sxm_t[:], xm_sb[:, ft, :], skip_sb[:, ft:ft + 1])
                nc.sync.dma_start(
                    out=sxm_o[ft * DH:(ft + 1) * DH, :], in_=sxm_t[:])

            # --- q/k/v projections (streamed weights) + gates
            qkv_outs = [q_o, k_o, v_o]
            for m in range(MT_QKV):
                w_sb = wpool.tile([DH, KT_IN, DH], BF16)
                nc.sync.dma_start(
                    out=w_sb,
                    in_=wqkvT[:, m * DH:(m + 1) * DH].rearrange("(kt p) m -> p kt m", p=DH))
                out_t = qkv_outs[m // FT]
                mf = m % FT
                ev_t = ev.tile([DH, TOK], BF16)
                for (n0, nn) in N_SL_OUT:
                    pt = ps.tile([DH, 512], F32)
                    for kt in range(KT_IN):
                        nc.tensor.matmul(
                            pt[:, :nn], w_sb[:, kt, :], xm_sb[:, kt, n0:n0 + nn],
                            start=(kt == 0), stop=(kt == KT_IN - 1))
                    if m % 2 == 0:
                        nc.scalar.copy(ev_t[:, n0:n0 + nn], pt[:, :nn])
                    else:
                        nc.vector.tensor_copy(ev_t[:, n0:n0 + nn], pt[:, :nn])
                nc.sync.dma_start(
                    out=out_t[mf * DH:(mf + 1) * DH, :], in_=ev_t[:])


            # --- up-projection og half + silu(x_og) before the (tiny) gates
            for m in range(FT):
                for (n0, nn) in N_SL_OUT:
                    pt = ps.tile([DH, 512], F32)
                    for kt in range(KT_UP):
                        nc.tensor.matmul(
                            pt[:, :nn],
                            wup_sb[:, kt, (FT + m) * DH:(FT + m + 1) * DH],
                            xt_sb[:, kt, KCONV - 1 + n0: KCONV - 1 + n0 + nn],
                            start=(kt == 0), stop=(kt == KT_UP - 1))
                    if m % 2 == 0:
                        nc.scalar.copy(xog_sb[:, m, n0:n0 + nn], pt[:, :nn])
                    else:
                        nc.vector.tensor_copy(xog_sb[:, m, n0:n0 + nn], pt[:, :nn])
                # g2 = silu(x_og) for this tile immediately
                sg2_t = cv.tile([DH, TOK], BF16, tag="sg2")
                nc.scalar.activation(sg2_t[:], xog_sb[:, m, :], AF.Sigmoid)
                g2_t = cv.tile([DH, TOK], BF16, tag="out")
                nc.vector.tensor_tensor(g2_t[:], xog_sb[:, m, :], sg2_t[:], OP.mult)
                nc.sync.dma_start(out=g2_o[m * DH:(m + 1) * DH, :], in_=g2_t[:])

            # gates: [16, TOK]
            wg_sb = const.tile([DH, KT_IN, 2 * NH], BF16)
            nc.sync.dma_start(out=wg_sb, in_=wgT[:].rearrange("(kt p) m -> p kt m", p=DH))
            for (n0, nn) in N_SL_OUT:
                pt = ps.tile([2 * NH, 512], F32)
                for kt in range(KT_IN):
                    nc.tensor.matmul(
                        pt[:, :nn], wg_sb[:, kt, :], xm_sb[:, kt, n0:n0 + nn],
                        start=(kt == 0), stop=(kt == KT_IN - 1))
                gv = ev.tile([2 * NH, 512], F32)
                nc.vector.tensor_copy(gv[:, :nn], pt[:, :nn])
                nc.sync.dma_start(out=gates_o[:, n0:n0 + nn], in_=gv[:, :nn])
    nc.compile()
    return nc


# ---------------------------------------------------------------- phase B ----
def build_phase_b():
    nc = new_nc()
    ins = {}
    outs = {}
    for u in range(NUNIT):
        # feat-major q and (esc*cch)-scaled k
        ins[f"qT{u}"] = nc.dram_tensor(f"qT{u}", [DH, S], BF16, kind="ExternalInput")
        ins[f"kTc{u}"] = nc.dram_tensor(f"kTc{u}", [DH, S], BF16, kind="ExternalInput")
        # token-major (p = token-in-chunk): [p, c, d] esc-scaled k, [p, c, e] = [v | 1]
        ins[f"kesc{u}"] = nc.dram_tensor(f"kesc{u}", [DH, NCH * DH], BF16, kind="ExternalInput")
        ins[f"vone{u}"] = nc.dram_tensor(f"vone{u}", [DH, NCH * NW], BF16, kind="ExternalInput")
        ins[f"dec{u}"] = nc.dram_tensor(f"dec{u}", [DH, NCH], F32, kind="ExternalInput")
        ins[f"e2{u}"] = nc.dram_tensor(f"e2{u}", [DH, NCH], F32, kind="ExternalInput")
        ins[f"e3{u}"] = nc.dram_tensor(f"e3{u}", [DH, NCH], F32, kind="ExternalInput")
        # h out in [p, c, e] layout
        outs[f"h{u}"] = nc.dram_tensor(f"h{u}", [DH, NCH * DH], BF16, kind="ExternalOutput")
    mask_i = nc.dram_tensor("mask_i", [DH, 4 * DH], BF16, kind="ExternalInput")

    UB = 4    # chunks per mm2/U-copy block
    SB = 4    # chunks per mm1/sp/num block
    NE = NW * NCH   # 8256 elements per shared ring buffer
    with tile.TileContext(nc) as tc, \
         tc.tile_pool(name="small", bufs=1) as small, \
         tc.tile_pool(name="sh", bufs=11) as sh, \
         tc.tile_pool(name="spb", bufs=2) as spb, \
         tc.tile_pool(name="work", bufs=2) as work, \
         tc.tile_pool(name="hst", bufs=2) as hst, \
         tc.tile_pool(name="ps1", bufs=2, space="PSUM") as ps1, \
         tc.tile_pool(name="psn", bufs=3, space="PSUM") as psn:
        if True:
            def ring(name):
                return sh.tile([DH, NE], BF16, tag="sh", name=name)

            mask_sb = small.tile([DH, SB, DH], BF16, name="mask")
            nc.sync.dma_start(
                out=mask_sb, in_=mask_i[:].rearrange("p (b l) -> p b l", b=SB))
            T = {}
            for u in range(NUNIT):
                T[u] = dict(
                    dec=small.tile([DH, NCH], F32, name=f"dec{u}"),
                    e2=small.tile([DH, NCH], F32, name=f"e2{u}"),
                    e3=small.tile([DH, NCH], F32, name=f"e3{u}"),
                )
                nc.sync.dma_start(out=T[u]['dec'], in_=ins[f"dec{u}"][:])
                nc.sync.dma_start(out=T[u]['e2'], in_=ins[f"e2{u}"][:])
                nc.sync.dma_start(out=T[u]['e3'], in_=ins[f"e3{u}"][:])
            # ring allocation order (12 bufs): late tensors wrap onto buffers
            # that die early (qT1 -> kesc0, kTc1 -> U0).
            T[0]['kesc'] = ring("kesc0")
            T[0]['U'] = ring("U0")
            T[0]['decf'] = ring("decf0")
            T[0]['Cs'] = ring("Cs0")
            T[1]['kesc'] = ring("kesc1")
            T[0]['vone'] = ring("vone0")
            T[0]['qT'] = ring("qT0")
            T[0]['kTc'] = ring("kTc0")
            T[1]['vone'] = ring("vone1")
            T[1]['U'] = ring("U1")
            T[1]['decf'] = ring("decf1")
            T[1]['Cs'] = ring("Cs1")
            T[1]['qT'] = ring("qT1")
            T[1]['kTc'] = ring("kTc1")
            # DMA issue order: u0 working set first, then u1
            HALF = NCH // 2
            for u in range(NUNIT):
                for half in range(2):
                    ks = slice(half * HALF * DH, (half + 1) * HALF * DH)
                    vs = slice(half * HALF * NW, (half + 1) * HALF * NW)
                    nc.sync.dma_start(out=T[u]['kesc'][:, ks],
                                      in_=ins[f"kesc{u}"][:, ks])
                    nc.sync.dma_start(out=T[u]['vone'][:, vs],
                                      in_=ins[f"vone{u}"][:, vs])
                nc.sync.dma_start(out=T[u]['qT'][:, :S], in_=ins[f"qT{u}"][:])
                nc.sync.dma_start(out=T[u]['kTc'][:, :S], in_=ins[f"kTc{u}"][:])

            def kescv(u):
                return T[u]['kesc'][:, :NCH * DH].rearrange("p (c d) -> p c d", c=NCH)

            def vonev(u):
                return T[u]['vone'][:].rearrange("p (c e) -> p c e", c=NCH)

            def Uv(u):
                return T[u]['U'][:].rearrange("p (e c) -> p e c", e=NW)

            def Csv(u):
                return T[u]['Cs'][:].rearrange("p (e c) -> p e c", e=NW)

            def stage_i(u):
                for cb in range(NCH // UB):
                    up = psn.tile([DH, UB, 256], F32, tag="nps")
                    for i in range(UB):
                        c = cb * UB + i
                        nc.tensor.matmul(
                            up[:, i, :NW], kescv(u)[:, c, :], vonev(u)[:, c, :],
                            start=True, stop=True)
                    # PSUM -> SBUF bf16, strided into [p, e, c] layout
                    nc.scalar.copy(
                        Uv(u)[:, :, cb * UB:(cb + 1) * UB].rearrange("p e c -> p c e"),
                        up[:, :, :NW])

            def scan(u):
                t = T[u]
                # state after chunk c at [:, e, c]; col c=0 of dec is 0 =>
                # resets the carried state at each e-boundary.
                dec_bc = t['dec'][:].unsqueeze(1).broadcast_to([DH, NW, NCH])
                decfv = t['decf'][:].rearrange("p (e c) -> p e c", e=NW)
                if u == 0:
                    nc.vector.tensor_copy(decfv, dec_bc)
                else:
                    nc.gpsimd.tensor_copy(decfv, dec_bc)
                nc.vector.tensor_tensor_scan(
                    t['Cs'][:], t['decf'][:], t['U'][:],
                    0.0, OP.mult, OP.add)

            def stage_ii(u):
                for cb in range(NCH // SB):
                    t = T[u]
                    # S matmuls for the block
                    sps = ps1.tile([DH, SB, DH], F32, tag="sps")
                    for i in range(SB):
                        c = cb * SB + i
                        csl = slice(c * LC, (c + 1) * LC)
                        nc.tensor.matmul(
                            sps[:, i, :], t['kTc'][:, csl], t['qT'][:, csl],
                            start=True, stop=True)
                    # Sp = S * mask (the esc/cch scaling is baked into kTc)
                    sp = spb.tile([DH, SB, DH], BF16, tag="sp")
                    nc.vector.tensor_tensor(sp[:], sps[:], mask_sb[:], OP.mult)
                    # num matmuls
                    nps = psn.tile([DH, SB, 256], F32, tag="nps")
                    for i in range(SB):
                        c = cb * SB + i
                        csl = slice(c * LC, (c + 1) * LC)
                        if c > 0:
                            nc.tensor.matmul(
                                nps[:, i, :NW], t['qT'][:, csl], Csv(u)[:, :, c - 1],
                                start=True, stop=False)
                        nc.tensor.matmul(
                            nps[:, i, :NW], sp[:, i, :], vonev(u)[:, c, :],
                            start=(c == 0), stop=True)
                    # den for the whole block: den = max(|raw|, e2) + e3
                    absd = work.tile([DH, SB], F32, tag="absd")
                    nc.scalar.activation(absd[:], nps[:, :, DH], AF.Abs)
                    den = work.tile([DH, SB], F32, tag="den")
                    nc.vector.tensor_tensor(
                        den[:], absd[:], t['e2'][:, cb * SB:(cb + 1) * SB], OP.max)
                    den2 = work.tile([DH, SB], F32, tag="den2")
                    nc.gpsimd.tensor_tensor(
                        den2[:], den[:], t['e3'][:, cb * SB:(cb + 1) * SB], OP.add)
                    rden = work.tile([DH, SB], F32, tag="rden")
                    nc.vector.reciprocal(rden[:], den2[:])
                    # h = num[:, :, :128] * rden (broadcast / per-chunk scale)
                    h_sb = hst.tile([DH, SB, DH], BF16, tag="h_sb")
                    if cb % 2 == 0:
                        nc.vector.tensor_tensor(
                            h_sb[:], nps[:, :, :DH],
                            rden[:].unsqueeze(2).broadcast_to([DH, SB, DH]), OP.mult)
                    else:
                        for i in range(SB):
                            nc.scalar.activation(
                                h_sb[:, i, :], nps[:, i, :DH], AF.Copy,
                                bias=0.0, scale=rden[:, i:i + 1])
                    nc.sync.dma_start(
                        out=outs[f"h{u}"][:, cb * SB * DH:(cb + 1) * SB * DH],
                        in_=h_sb[:])

            stage_i(0)
            stage_i(1)
            scan(0)
            stage_ii(0)
            scan(1)
            stage_ii(1)
    nc.compile()
    return nc


# ---------------------------------------------------------------- phase C ----
def build_phase_c():
    nc = new_nc()
    h_i = nc.dram_tensor("h_i", [INNER, TOK], BF16, kind="ExternalInput")
    sxm_i = nc.dram_tensor("sxm_i", [INNER, TOK], BF16, kind="ExternalInput")
    g2_i = nc.dram_tensor("g2_i", [INNER, TOK], BF16, kind="ExternalInput")
    wdT = nc.dram_tensor("wdT", [INNER, D], BF16, kind="ExternalInput")
    out_o = nc.dram_tensor("out_o", [D, TOK], BF16, kind="ExternalOutput")

    FT = INNER // DH   # 8
    MT = D // DH       # 4
    HK = TOK // 2      # token half
    with tile.TileContext(nc) as tc, \
         tc.tile_pool(name="big", bufs=1) as big, \
         tc.tile_pool(name="ev", bufs=4) as ev, \
         tc.tile_pool(name="ps", bufs=4, space="PSUM") as ps:
        if True:
            wd_sb = big.tile([DH, FT, D], BF16)
            nc.sync.dma_start(out=wd_sb, in_=wdT[:].rearrange("(ft p) m -> p ft m", p=DH))
            h_sb = big.tile([DH, FT, TOK], BF16)
            sxm_sb = big.tile([DH, FT, TOK], BF16)
            g2_sb = big.tile([DH, FT, TOK], BF16)
            hg_sb = big.tile([DH, FT, TOK], BF16)
            for half in range(2):
                tsl = slice(half * HK, (half + 1) * HK)
                nc.sync.dma_start(
                    out=h_sb[:, :, tsl],
                    in_=h_i[:, tsl].rearrange("(ft p) t -> p ft t", p=DH))
                nc.sync.dma_start(
                    out=sxm_sb[:, :, tsl],
                    in_=sxm_i[:, tsl].rearrange("(ft p) t -> p ft t", p=DH))
                nc.sync.dma_start(
                    out=g2_sb[:, :, tsl],
                    in_=g2_i[:, tsl].rearrange("(ft p) t -> p ft t", p=DH))
                for ft in range(FT):
                    hs = ev.tile([DH, HK], BF16, tag="hs")
                    nc.vector.tensor_tensor(
                        hs[:], h_sb[:, ft, tsl], sxm_sb[:, ft, tsl], OP.add)
                    nc.vector.tensor_tensor(
                        hg_sb[:, ft, tsl], hs[:], g2_sb[:, ft, tsl], OP.mult)
                for m in range(MT):
                    for ns in range(2):
                        n0 = half * HK + ns * 512
                        pt = ps.tile([DH, 512], F32)
                        for kt in range(FT):
                            nc.tensor.matmul(
                                pt[:], wd_sb[:, kt, m * DH:(m + 1) * DH],
                                hg_sb[:, kt, n0:n0 + 512],
                                start=(kt == 0), stop=(kt == FT - 1))
                        ot = ev.tile([DH, 512], BF16, tag="ot")
                        if m % 2 == 0:
                            nc.scalar.copy(ot[:], pt[:])
                        else:
                            nc.vector.tensor_copy(ot[:], pt[:])
                        nc.sync.dma_start(
                            out=out_o[m * DH:(m + 1) * DH, n0:n0 + 512], in_=ot[:])
    nc.compile()
    return nc


# ------------------------------------------------------------- host glue ----
def host_gate_math(i_pre, f_pre):
    """i_pre, f_pre: [B, NH, S] f32.  Returns dict of f32 arrays.

    Exports (per b, h):
      esc   [S]        exp(a_j - m_new(chunk))            per-token k scale
      cch   [NCH]      exp(m_new - ms - scaG)             per-chunk Sp scale
      dec   [NCH]      exp(scaG + ms - m_new), col0 = 0   scan multiplier
      e2,e3 [NCH, LC]  den clamp terms (qk_scale folded)
    """
    i_pre = i_pre.astype(np.float64)
    f_pre = f_pre.astype(np.float64)
    vecI = np.log(1.0 / (1.0 + np.exp(-i_pre)) + EPS)
    vecF = np.log(1.0 / (1.0 + np.exp(-f_pre)) + EPS)
    Ic = vecI.reshape(B, NH, NCH, LC)
    Fc = vecF.reshape(B, NH, NCH, LC)
    vecB = np.cumsum(Fc, axis=-1)
    scaG = vecB[..., -1]
    vecA = scaG[..., None] - vecB + Ic

    ms = np.zeros((B, NH, NCH))
    dec = np.zeros((B, NH, NCH))
    m_new_arr = np.zeros((B, NH, NCH))
    m = np.zeros((B, NH))
    for c in range(NCH):
        amax = vecA[:, :, c, :].max(-1)
        m_new = np.maximum(scaG[:, :, c] + m, amax)
        ms[:, :, c] = m
        dec[:, :, c] = np.exp(scaG[:, :, c] + m - m_new)
        m_new_arr[:, :, c] = m_new
        m = m_new
    escale = np.exp(vecA - m_new_arr[..., None])          # [B,NH,NCH,LC]
    cch = np.exp(m_new_arr - ms - scaG)                   # [B,NH,NCH]

    mask = np.tril(np.ones((LC, LC), bool))
    logD = vecB[..., :, None] - vecB[..., None, :] + Ic[..., None, :]
    logD = np.where(mask, logD, -np.inf)
    m_intra = logD.max(-1)
    m_comb = np.maximum(vecB + ms[..., None], m_intra)
    e2 = np.exp(-vecB - ms[..., None]) / QK_SCALE
    e3 = EPS * np.exp(m_comb - vecB - ms[..., None]) / QK_SCALE
    dec0 = dec.copy()
    dec0[:, :, 0] = 0.0
    return dict(
        esc=escale.reshape(B, NH, S).astype(np.float32),
        cch=cch.astype(np.float32),
        dec=dec0.astype(np.float32),
        e2=e2.astype(np.float32), e3=e3.astype(np.float32))


def prep_weights(W_up, Wq, Wk, Wv, W_ig, W_fg, conv_w, conv_b, skip, W_down):
    """Host-side weight packing (same for all cores)."""
    FT = INNER // DH
    wupT = _bf(W_up.T)                                         # [512, 2048]
    wqkvT = _bf(np.concatenate([Wq.T, Wk.T, Wv.T], axis=1))    # [1024, 3072]
    wgT = _bf(np.concatenate([W_ig.T, W_fg.T], axis=1))        # [1024, 16]
    convw = np.ascontiguousarray(
        conv_w.reshape(FT, DH, KCONV).transpose(1, 0, 2).reshape(DH, FT * KCONV)
    ).astype(np.float32)
    convb = np.ascontiguousarray(conv_b.reshape(FT, DH).T).astype(np.float32)
    skip_p = np.ascontiguousarray(skip.reshape(FT, DH).T).astype(np.float32)
    wdT = _bf(W_down.T)                                        # [1024, 512]
    mask4 = _bf(np.tile(np.tril(np.ones((DH, DH), np.float32)), (1, 4)))
    return dict(wupT=wupT, wqkvT=wqkvT, wgT=wgT, convw=convw, convb=convb,
                skip_p=skip_p, wdT=wdT, mask4=mask4)


def build_a_inmaps(x, wp):
    """Per-core phase A input maps.  Core c = (b=c//4, quarter=c%4)."""
    in_maps = []
    for c in range(8):
        b, qt = c // 4, c % 4
        s0 = qt * TOK
        xs = x[b, :, :].T                                       # [512, S] view
        if s0 == 0:
            xt = np.concatenate([np.zeros((D, KCONV - 1), np.float32),
                                 xs[:, :TOK]], axis=1)
        else:
            xt = xs[:, s0 - (KCONV - 1): s0 + TOK]
        in_maps.append(dict(
            xt=_bf(xt), wupT=wp['wupT'], wqkvT=wp['wqkvT'], wgT=wp['wgT'],
            convw=wp['convw'], convb=wp['convb'], skip_i=wp['skip_p']))
    return in_maps


def assemble_a_outputs(a_results, b_ig, b_fg):
    """Concatenate per-core phase A outputs into full feature-major tensors."""
    def cat(name):
        return np.stack([
            np.concatenate([a_results[b * 4 + qt][name] for qt in range(4)], axis=1)
            for b in range(B)])
    q_t, k_t, v_t = cat('q_o'), cat('k_o'), cat('v_o')          # [B, INNER, S] bf16
    sxm_t, g2_t = cat('sxm_o'), cat('g2_o')
    gates = cat('gates_o').astype(np.float32)                   # [B, 16, S]
    i_pre = gates[:, :NH, :] + np.asarray(b_ig, np.float32)[None, :, None]
    f_pre = gates[:, NH:, :] + np.asarray(b_fg, np.float32)[None, :, None]
    return q_t, k_t, v_t, sxm_t, g2_t, i_pre, f_pre


def build_b_inmaps(q_t, k_t, v_t, g, wp):
    """Per-core phase B inputs.  Core c handles units (b, 2h) where
    b = c // 4, heads (2*(c%4), 2*(c%4)+1)."""
    in_maps = []
    for c in range(8):
        b, hp = c // 4, c % 4
        m = {'mask_i': wp['mask4']}
        for u in range(NUNIT):
            h = 2 * hp + u
            rs = slice(h * DH, (h + 1) * DH)
            kf = k_t[b, rs, :].astype(np.float32)               # [128, S] feat-major
            esc = g['esc'][b, h]                                # [S]
            cch_tok = np.repeat(g['cch'][b, h], LC)             # [S]
            m[f"qT{u}"] = np.ascontiguousarray(q_t[b, rs, :])
            m[f"kTc{u}"] = _bf(kf * (esc * cch_tok)[None, :])
            # token-major [p, c, d] / [p, c, e]
            kesc = (kf.T * esc[:, None]).reshape(NCH, LC, DH)
            m[f"kesc{u}"] = _bf(kesc.transpose(1, 0, 2).reshape(DH, NCH * DH))
            vone = np.empty((NCH, LC, NW), np.float32)
            vone[:, :, :DH] = v_t[b, rs, :].astype(np.float32).T.reshape(NCH, LC, DH)
            vone[:, :, DH] = 1.0
            m[f"vone{u}"] = _bf(vone.transpose(1, 0, 2).reshape(DH, NCH * NW))
            m[f"dec{u}"] = np.ascontiguousarray(
                np.broadcast_to(g['dec'][b, h][None, :], (DH, NCH)).astype(np.float32))
            m[f"e2{u}"] = np.ascontiguousarray(g['e2'][b, h].T.astype(np.float32))
            m[f"e3{u}"] = np.ascontiguousarray(g['e3'][b, h].T.astype(np.float32))
        in_maps.append(m)
    return in_maps


def build_c_inmaps(b_results, sxm_t, g2_t, wp):
    """Assemble h from phase B [p, c, e] layouts into feature-major h_t,
    then per-core phase C inputs."""
    h_t = np.empty((B, INNER, S), dtype=sxm_t.dtype)
    for c in range(8):
        b, hp = c // 4, c % 4
        for u in range(NUNIT):
            h = 2 * hp + u
            # [p, c, e] -> [e, c*LC + p]
            harr = np.asarray(b_results[c][f"h{u}"]).reshape(DH, NCH, DH)
            h_t[b, h * DH:(h + 1) * DH, :] = (
                harr.transpose(2, 1, 0).reshape(DH, S))
    in_maps = []
    for c in range(8):
        b, qt = c // 4, c % 4
        ts = slice(qt * TOK, (qt + 1) * TOK)
        in_maps.append(dict(
            h_i=np.ascontiguousarray(h_t[b, :, ts]),
            sxm_i=np.ascontiguousarray(sxm_t[b, :, ts]),
            g2_i=np.ascontiguousarray(g2_t[b, :, ts]),
            wdT=wp['wdT']))
    return in_maps


def assemble_output(c_results):
    out = np.empty((B, S, D), np.float32)
    for c in range(8):
        b, qt = c // 4, c % 4
        out[b, qt * TOK:(qt + 1) * TOK, :] = \
            np.asarray(c_results[c]['out_o']).astype(np.float32).T
    return out


# ------------------------------------------------------------------ entry ----
from concourse.bass_utils import run_bass_kernel_spmd as _run_spmd

_CACHE = {}


def _programs():
    if 'a' not in _CACHE:
        _CACHE['a'] = build_phase_a()
        _CACHE['b'] = build_phase_b()
        _CACHE['c'] = build_phase_c()
    return _CACHE['a'], _CACHE['b'], _CACHE['c']


def kernel(x, W_up, Wq, Wk, Wv, W_ig, b_ig, W_fg, b_fg, conv_w, conv_b, skip,
           W_down):
    x = np.asarray(x, np.float32)
    nc_a, nc_b, nc_c = _programs()
    cores = list(range(8))
    wp = prep_weights(W_up, Wq, Wk, Wv, W_ig, W_fg, conv_w, conv_b, skip, W_down)
    a_maps = build_a_inmaps(x, wp)
    ra = _run_spmd(nc_a, a_maps, core_ids=cores).results
    q_t, k_t, v_t, sxm_t, g2_t, i_pre, f_pre = assemble_a_outputs(ra, b_ig, b_fg)
    g = host_gate_math(i_pre, f_pre)
    b_maps = build_b_inmaps(q_t, k_t, v_t, g, wp)
    rb = _run_spmd(nc_b, b_maps, core_ids=cores).results
    c_maps = build_c_inmaps(rb, sxm_t, g2_t, wp)
    rc = _run_spmd(nc_c, c_maps, core_ids=cores).results
    return assemble_output(rc)


# revision 28
# speedup vs baseline: 1.2760x; 1.0556x over previous
"""Trainium2 Bass kernel for nn_ChunkwiseMLSTM (B=2, S=8192, D=512, INNER=1024, NH=8).

kernel(**inputs) -> np.ndarray [2, 8192, 512] f32.

Three SPMD launches on 8 NeuronCores:
  A: token-sharded projections (up-proj, DVE causal conv+SiLU, q/k/v, gate
     pre-acts, pre-gated skip*xm and silu(x_og))
  B: head-sharded chunkwise mLSTM (LC=128 chunks), batched:
     - stage i: all inter-chunk state updates U_c = kesc_c^T @ [v|1] (PE),
       PSUM->SBUF bf16 copies (ACT), then ONE tensor_tensor_scan per unit
       (Pool) computes every chunk-boundary state [C|n] with f32 carry.
     - stage ii: per 4-chunk blocks: S_mat matmuls (PE), mask-mult (DVE),
       num matmuls against scanned states (PE), batched den (ACT/DVE),
       per-chunk h scale-copy (ACT/DVE alternating).
  C: token-sharded output gating + down-projection, token-halved pipeline.
Host between launches: gate-derived scan scalars (O(B*NH*S)), weight
pre-transposition, resharding.
"""
import os
os.environ.setdefault("JAX_COMPILATION_CACHE_DIR",
                      os.path.expanduser("~/.cache/jax_bass_cache"))
os.environ.setdefault("JAX_PERSISTENT_CACHE_MIN_ENTRY_SIZE_BYTES", "0")
os.environ.setdefault("JAX_PERSISTENT_CACHE_MIN_COMPILE_TIME_SECS", "0")

import sys
if '/opt/trn_rl_repo' not in sys.path:
    sys.path.insert(0, '/opt/trn_rl_repo')

import numpy as np
import ml_dtypes

import concourse.bass as bass
import concourse.tile as tile
from concourse import mybir, bacc

F32 = mybir.dt.float32
BF16 = mybir.dt.bfloat16
AF = mybir.ActivationFunctionType
OP = mybir.AluOpType

B, S, D = 2, 8192, 512
INNER, NH, KCONV = 1024, 8, 4
DH = 128
EPS = 1e-6
LC = 128           # chunk length used on device (math is chunk-size invariant)
NCH = S // LC      # 64
NW = 129           # [C | n] state width
QK_SCALE = DH ** -0.5
TOK = S // 4       # tokens per core in phases A/C = 2048
TH = TOK + (KCONV - 1)   # 2051 with conv halo
NUNIT = 2          # (b,h) units per core in phase B


def _bf(x):
    return np.ascontiguousarray(np.asarray(x, np.float32).astype(ml_dtypes.bfloat16))


def new_nc():
    return bacc.Bacc(None, target_bir_lowering=False, debug=False)


# ---------------------------------------------------------------- phase A ----
def build_phase_a():
    nc = new_nc()
    xt = nc.dram_tensor("xt", [D, TH], BF16, kind="ExternalInput")            # x[b].T slice (halo)
    wupT = nc.dram_tensor("wupT", [D, 2 * INNER], BF16, kind="ExternalInput")  # W_up.T
    wqkvT = nc.dram_tensor("wqkvT", [INNER, 3 * INNER], BF16, kind="ExternalInput")
    wgT = nc.dram_tensor("wgT", [INNER, 2 * NH], BF16, kind="ExternalInput")   # [Wig.T | Wfg.T]
    convw = nc.dram_tensor("convw", [DH, (INNER // DH) * KCONV], F32, kind="ExternalInput")
    convb = nc.dram_tensor("convb", [DH, INNER // DH], F32, kind="ExternalInput")
    skip_i = nc.dram_tensor("skip_i", [DH, INNER // DH], F32, kind="ExternalInput")

    q_o = nc.dram_tensor("q_o", [INNER, TOK], BF16, kind="ExternalOutput")
    k_o = nc.dram_tensor("k_o", [INNER, TOK], BF16, kind="ExternalOutput")
    v_o = nc.dram_tensor("v_o", [INNER, TOK], BF16, kind="ExternalOutput")
    sxm_o = nc.dram_tensor("sxm_o", [INNER, TOK], BF16, kind="ExternalOutput")   # skip * xm
    g2_o = nc.dram_tensor("g2_o", [INNER, TOK], BF16, kind="ExternalOutput")     # silu(x_og)
    gates_o = nc.dram_tensor("gates_o", [2 * NH, TOK], F32, kind="ExternalOutput")

    KT_UP = D // DH          # 4 k-tiles for up-proj
    FT = INNER // DH         # 8 feature tiles of the mlstm half
    MT_UP = 2 * FT           # 16 m-tiles of up-proj
    KT_IN = INNER // DH      # 8 k-tiles over INNER
    MT_QKV = 3 * FT          # 24
    N_SL = [(i * 512, min(512, TH - i * 512)) for i in range((TH + 511) // 512)]
    N_SL_OUT = [(i * 512, 512) for i in range(TOK // 512)]

    with tile.TileContext(nc) as tc, \
         tc.tile_pool(name="const", bufs=1) as const, \
         tc.tile_pool(name="big", bufs=1) as big, \
         tc.tile_pool(name="wpool", bufs=3) as wpool, \
         tc.tile_pool(name="ev", bufs=4) as ev, \
         tc.tile_pool(name="cv", bufs=3) as cv, \
         tc.tile_pool(name="ps", bufs=4, space="PSUM") as ps:
        if True:
            # --- load x and W_up
            wup_sb = big.tile([DH, KT_UP, 2 * INNER], BF16)
            nc.sync.dma_start(out=wup_sb[:, :, :INNER],
                              in_=wupT[:, :INNER].rearrange("(kt p) m -> p kt m", p=DH))
            xt_sb = big.tile([DH, KT_UP, TH], BF16)
            for kt in range(KT_UP):
                nc.sync.dma_start(out=xt_sb[:, kt, :],
                                  in_=xt[kt * DH:(kt + 1) * DH, :])
            nc.sync.dma_start(out=wup_sb[:, :, INNER:],
                              in_=wupT[:, INNER:].rearrange("(kt p) m -> p kt m", p=DH))
            convw_sb = const.tile([DH, FT, KCONV], F32)
            nc.sync.dma_start(out=convw_sb, in_=convw[:].rearrange("p (ft t) -> p ft t", ft=FT))
            convb_sb = const.tile([DH, FT], F32)
            nc.sync.dma_start(out=convb_sb, in_=convb[:])
            skip_sb = const.tile([DH, FT], F32)
            nc.sync.dma_start(out=skip_sb, in_=skip_i[:])

            # --- up-projection mlstm half: x_pre (bf16, with halo, kept)
            xpre_sb = big.tile([DH, FT, TH], BF16)
            xog_sb = big.tile([DH, FT, TOK], BF16)
            for m in range(FT):
                for (n0, nn) in N_SL:
                    pt = ps.tile([DH, 512], F32)
                    for kt in range(KT_UP):
                        nc.tensor.matmul(
                            pt[:, :nn],
                            wup_sb[:, kt, m * DH:(m + 1) * DH],
                            xt_sb[:, kt, n0: n0 + nn],
                            start=(kt == 0), stop=(kt == KT_UP - 1))
                    nc.scalar.copy(xpre_sb[:, m, n0:n0 + nn], pt[:, :nn])

            # --- causal depthwise conv + SiLU, token halves, alternating
            # DVE/Pool per feature tile so the two engines run in parallel.
            # y[f, t] = sum_tau w[f, tau] * xpre[f, t + tau]
            xm_sb = big.tile([DH, FT, TOK], BF16)
            HT = TOK // 2
            for half in range(2):
                h0 = half * HT
                for ft in range(FT):
                    ce = nc.vector
                    xe = nc.vector
                    a = None
                    for tau in range(KCONV):
                        an = cv.tile([DH, HT], BF16, tag="acc")
                        if tau == 0:
                            ce.tensor_scalar_mul(
                                an[:], xpre_sb[:, ft, h0:h0 + HT],
                                convw_sb[:, ft, 0:1])
                        else:
                            ce.scalar_tensor_tensor(
                                an[:], xpre_sb[:, ft, h0 + tau:h0 + tau + HT],
                                convw_sb[:, ft, tau:tau + 1], a[:],
                                OP.mult, OP.add)
                        a = an
                    # silu(y+b) = (y+b) * sigmoid(y+b)
                    sg_t = cv.tile([DH, HT], BF16, tag="sg")
                    nc.scalar.activation(
                        sg_t[:], a[:], AF.Sigmoid,
                        bias=convb_sb[:, ft:ft + 1], scale=1.0)
                    xe.scalar_tensor_tensor(
                        xm_sb[:, ft, h0:h0 + HT], a[:], convb_sb[:, ft:ft + 1],
                        sg_t[:], OP.add, OP.mult)


            # --- up-projection og half: fills the PE-idle conv window
            # (copies on ACT; DVE is busy with the conv chains)
            for m in range(FT):
                for (n0, nn) in N_SL_OUT:
                    pt = ps.tile([DH, 512], F32)
                    for kt in range(KT_UP):
                        nc.tensor.matmul(
                            pt[:, :nn],
                            wup_sb[:, kt, (FT + m) * DH:(FT + m + 1) * DH],
                            xt_sb[:, kt, KCONV - 1 + n0: KCONV - 1 + n0 + nn],
                            start=(kt == 0), stop=(kt == KT_UP - 1))
                    nc.scalar.copy(xog_sb[:, m, n0:n0 + nn], pt[:, :nn])

            # --- skip * xm (DVE slack while PE runs qkv)
            for ft in range(FT):
                sxm_t = cv.tile([DH, TOK], BF16, tag="out")
                nc.vector.tensor_scalar_mul(sxm_t[:], xm_sb[:, ft, :], skip_sb[:, ft:ft + 1])
                nc.sync.dma_start(
                    out=sxm_o[ft * DH:(ft + 1) * DH, :], in_=sxm_t[:])

            # --- q/k/v projections (streamed weights) + gates
            qkv_outs = [q_o, k_o, v_o]
            for m in range(MT_QKV):
                w_sb = wpool.tile([DH, KT_IN, DH], BF16)
                nc.sync.dma_start(
                    out=w_sb,
                    in_=wqkvT[:, m * DH:(m + 1) * DH].rearrange("(kt p) m -> p kt m", p=DH))
                out_t = qkv_outs[m // FT]
                mf = m % FT
                ev_t = ev.tile([DH, TOK], BF16)
                for (n0, nn) in N_SL_OUT:
                    pt = ps.tile([DH, 512], F32)
                    for kt in range(KT_IN):
                        nc.tensor.matmul(
                            pt[:, :nn], w_sb[:, kt, :], xm_sb[:, kt, n0:n0 + nn],
                            start=(kt == 0), stop=(kt == KT_IN - 1))
                    if m % 2 == 0:
                        nc.scalar.copy(ev_t[:, n0:n0 + nn], pt[:, :nn])
                    else:
                        nc.vector.tensor_copy(ev_t[:, n0:n0 + nn], pt[:, :nn])
                nc.sync.dma_start(
                    out=out_t[mf * DH:(mf + 1) * DH, :], in_=ev_t[:])


            # gates: [16, TOK]
            wg_sb = const.tile([DH, KT_IN, 2 * NH], BF16)
            nc.sync.dma_start(out=wg_sb, in_=wgT[:].rearrange("(kt p) m -> p kt m", p=DH))
            for (n0, nn) in N_SL_OUT:
                pt = ps.tile([2 * NH, 512], F32)
                for kt in range(KT_IN):
                    nc.tensor.matmul(
                        pt[:, :nn], wg_sb[:, kt, :], xm_sb[:, kt, n0:n0 + nn],
                        start=(kt == 0), stop=(kt == KT_IN - 1))
                gv = ev.tile([2 * NH, 512], F32)
                nc.vector.tensor_copy(gv[:, :nn], pt[:, :nn])
                nc.sync.dma_start(out=gates_o[:, n0:n0 + nn], in_=gv[:, :nn])

            # --- g2 = silu(x_og) tail (overlaps late qkv/gates)
            for m in range(FT):
                sg2_t = cv.tile([DH, TOK], BF16, tag="sg2")
                nc.scalar.activation(sg2_t[:], xog_sb[:, m, :], AF.Sigmoid)
                g2_t = cv.tile([DH, TOK], BF16, tag="out")
                nc.vector.tensor_tensor(g2_t[:], xog_sb[:, m, :], sg2_t[:], OP.mult)
                nc.sync.dma_start(out=g2_o[m * DH:(m + 1) * DH, :], in_=g2_t[:])
    nc.compile()
    return nc


# ---------------------------------------------------------------- phase B ----
def build_phase_b():
    nc = new_nc()
    ins = {}
    outs = {}
    for u in range(NUNIT):
        # feat-major q and (esc*cch)-scaled k
        ins[f"qT{u}"] = nc.dram_tensor(f"qT{u}", [DH, S], BF16, kind="ExternalInput")
        ins[f"kTc{u}"] = nc.dram_tensor(f"kTc{u}", [DH, S], BF16, kind="ExternalInput")
        # token-major (p = token-in-chunk): [p, c, d] esc-scaled k, [p, c, e] = [v | 1]
        ins[f"kesc{u}"] = nc.dram_tensor(f"kesc{u}", [DH, NCH * DH], BF16, kind="ExternalInput")
        ins[f"vone{u}"] = nc.dram_tensor(f"vone{u}", [DH, NCH * NW], BF16, kind="ExternalInput")
        ins[f"dec{u}"] = nc.dram_tensor(f"dec{u}", [DH, NCH], F32, kind="ExternalInput")
        ins[f"e2{u}"] = nc.dram_tensor(f"e2{u}", [DH, NCH], F32, kind="ExternalInput")
        ins[f"e3{u}"] = nc.dram_tensor(f"e3{u}", [DH, NCH], F32, kind="ExternalInput")
        # h out in [p, c, e] layout
        outs[f"h{u}"] = nc.dram_tensor(f"h{u}", [DH, NCH * DH], BF16, kind="ExternalOutput")
    mask_i = nc.dram_tensor("mask_i", [DH, 4 * DH], BF16, kind="ExternalInput")

    UB = 4    # chunks per mm2/U-copy block
    SB = 4    # chunks per mm1/sp/num block
    NE = NW * NCH   # 8256 elements per shared ring buffer
    with tile.TileContext(nc) as tc, \
         tc.tile_pool(name="small", bufs=1) as small, \
         tc.tile_pool(name="sh", bufs=11) as sh, \
         tc.tile_pool(name="spb", bufs=2) as spb, \
         tc.tile_pool(name="work", bufs=2) as work, \
         tc.tile_pool(name="hst", bufs=2) as hst, \
         tc.tile_pool(name="ps1", bufs=2, space="PSUM") as ps1, \
         tc.tile_pool(name="psn", bufs=3, space="PSUM") as psn:
        if True:
            def ring(name):
                return sh.tile([DH, NE], BF16, tag="sh", name=name)

            mask_sb = small.tile([DH, SB, DH], BF16, name="mask")
            nc.sync.dma_start(
                out=mask_sb, in_=mask_i[:].rearrange("p (b l) -> p b l", b=SB))
            T = {}
            for u in range(NUNIT):
                T[u] = dict(
                    dec=small.tile([DH, NCH], F32, name=f"dec{u}"),
                    e2=small.tile([DH, NCH], F32, name=f"e2{u}"),
                    e3=small.tile([DH, NCH], F32, name=f"e3{u}"),
                )
                nc.sync.dma_start(out=T[u]['dec'], in_=ins[f"dec{u}"][:])
                nc.sync.dma_start(out=T[u]['e2'], in_=ins[f"e2{u}"][:])
                nc.sync.dma_start(out=T[u]['e3'], in_=ins[f"e3{u}"][:])
            # ring allocation order (12 bufs): late tensors wrap onto buffers
            # that die early (qT1 -> kesc0, kTc1 -> U0).
            T[0]['kesc'] = ring("kesc0")
            T[0]['U'] = ring("U0")
            T[0]['decf'] = ring("decf0")
            T[0]['Cs'] = ring("Cs0")
            T[1]['kesc'] = ring("kesc1")
            T[0]['vone'] = ring("vone0")
            T[0]['qT'] = ring("qT0")
            T[0]['kTc'] = ring("kTc0")
            T[1]['vone'] = ring("vone1")
            T[1]['U'] = ring("U1")
            T[1]['decf'] = ring("decf1")
            T[1]['Cs'] = ring("Cs1")
            T[1]['qT'] = ring("qT1")
            T[1]['kTc'] = ring("kTc1")
            # DMA issue order: u0 working set first, then u1
            HALF = NCH // 2
            for u in range(NUNIT):
                for half in range(2):
                    ks = slice(half * HALF * DH, (half + 1) * HALF * DH)
                    vs = slice(half * HALF * NW, (half + 1) * HALF * NW)
                    nc.sync.dma_start(out=T[u]['kesc'][:, ks],
                                      in_=ins[f"kesc{u}"][:, ks])
                    nc.sync.dma_start(out=T[u]['vone'][:, vs],
                                      in_=ins[f"vone{u}"][:, vs])
                nc.sync.dma_start(out=T[u]['qT'][:, :S], in_=ins[f"qT{u}"][:])
                nc.sync.dma_start(out=T[u]['kTc'][:, :S], in_=ins[f"kTc{u}"][:])

            def kescv(u):
                return T[u]['kesc'][:, :NCH * DH].rearrange("p (c d) -> p c d", c=NCH)

            def vonev(u):
                return T[u]['vone'][:].rearrange("p (c e) -> p c e", c=NCH)

            def Uv(u):
                return T[u]['U'][:].rearrange("p (e c) -> p e c", e=NW)

            def Csv(u):
                return T[u]['Cs'][:].rearrange("p (e c) -> p e c", e=NW)

            def stage_i(u):
                for cb in range(NCH // UB):
                    up = psn.tile([DH, UB, 256], F32, tag="nps")
                    for i in range(UB):
                        c = cb * UB + i
                        nc.tensor.matmul(
                            up[:, i, :NW], kescv(u)[:, c, :], vonev(u)[:, c, :],
                            start=True, stop=True)
                    # PSUM -> SBUF bf16, strided into [p, e, c] layout
                    nc.scalar.copy(
                        Uv(u)[:, :, cb * UB:(cb + 1) * UB].rearrange("p e c -> p c e"),
                        up[:, :, :NW])

            def scan(u):
                t = T[u]
                # state after chunk c at [:, e, c]; col c=0 of dec is 0 =>
                # resets the carried state at each e-boundary.
                dec_bc = t['dec'][:].unsqueeze(1).broadcast_to([DH, NW, NCH])
                decfv = t['decf'][:].rearrange("p (e c) -> p e c", e=NW)
                if u == 0:
                    nc.vector.tensor_copy(decfv, dec_bc)
                else:
                    nc.gpsimd.tensor_copy(decfv, dec_bc)
                nc.vector.tensor_tensor_scan(
                    t['Cs'][:], t['decf'][:], t['U'][:],
                    0.0, OP.mult, OP.add)

            def stage_ii_block(u, cb):
                if True:
                    t = T[u]
                    # S matmuls for the block
                    sps = ps1.tile([DH, SB, DH], F32, tag="sps")
                    for i in range(SB):
                        c = cb * SB + i
                        csl = slice(c * LC, (c + 1) * LC)
                        nc.tensor.matmul(
                            sps[:, i, :], t['kTc'][:, csl], t['qT'][:, csl],
                            start=True, stop=True)
                    # Sp = S * mask (the esc/cch scaling is baked into kTc)
                    sp = spb.tile([DH, SB, DH], BF16, tag="sp")
                    nc.vector.tensor_tensor(sp[:], sps[:], mask_sb[:], OP.mult)
                    # num matmuls
                    nps = psn.tile([DH, SB, 256], F32, tag="nps")
                    for i in range(SB):
                        c = cb * SB + i
                        csl = slice(c * LC, (c + 1) * LC)
                        if c > 0:
                            nc.tensor.matmul(
                                nps[:, i, :NW], t['qT'][:, csl], Csv(u)[:, :, c - 1],
                                start=True, stop=False)
                        nc.tensor.matmul(
                            nps[:, i, :NW], sp[:, i, :], vonev(u)[:, c, :],
                            start=(c == 0), stop=True)
                    # den for the whole block: den = max(|raw|, e2) + e3
                    absd = work.tile([DH, SB], F32, tag="absd")
                    nc.scalar.activation(absd[:], nps[:, :, DH], AF.Abs)
                    den = work.tile([DH, SB], F32, tag="den")
                    nc.vector.tensor_tensor(
                        den[:], absd[:], t['e2'][:, cb * SB:(cb + 1) * SB], OP.max)
                    den2 = work.tile([DH, SB], F32, tag="den2")
                    nc.gpsimd.tensor_tensor(
                        den2[:], den[:], t['e3'][:, cb * SB:(cb + 1) * SB], OP.add)
                    rden = work.tile([DH, SB], F32, tag="rden")
                    nc.vector.reciprocal(rden[:], den2[:])
                    # h = num[:, :, :128] * rden (broadcast / per-chunk scale)
                    h_sb = hst.tile([DH, SB, DH], BF16, tag="h_sb")
                    if cb % 2 == 0:
                        nc.vector.tensor_tensor(
                            h_sb[:], nps[:, :, :DH],
                            rden[:].unsqueeze(2).broadcast_to([DH, SB, DH]), OP.mult)
                    else:
                        for i in range(SB):
                            nc.scalar.activation(
                                h_sb[:, i, :], nps[:, i, :DH], AF.Copy,
                                bias=0.0, scale=rden[:, i:i + 1])
                    nc.sync.dma_start(
                        out=outs[f"h{u}"][:, cb * SB * DH:(cb + 1) * SB * DH],
                        in_=h_sb[:])

            stage_i(0)
            stage_i(1)
            scan(0)
            for cb in range(NCH // SB):
                stage_ii_block(0, cb)
            scan(1)
            for cb in range(NCH // SB):
                stage_ii_block(1, cb)
    nc.compile()
    return nc


# ---------------------------------------------------------------- phase C ----
def build_phase_c():
    nc = new_nc()
    h_i = nc.dram_tensor("h_i", [INNER, TOK], BF16, kind="ExternalInput")
    sxm_i = nc.dram_tensor("sxm_i", [INNER, TOK], BF16, kind="ExternalInput")
    g2_i = nc.dram_tensor("g2_i", [INNER, TOK], BF16, kind="ExternalInput")
    wdT = nc.dram_tensor("wdT", [INNER, D], BF16, kind="ExternalInput")
    out_o = nc.dram_tensor("out_o", [D, TOK], BF16, kind="ExternalOutput")

    FT = INNER // DH   # 8
    MT = D // DH       # 4
    HK = TOK // 2      # token half
    with tile.TileContext(nc) as tc, \
         tc.tile_pool(name="big", bufs=1) as big, \
         tc.tile_pool(name="ev", bufs=4) as ev, \
         tc.tile_pool(name="ps", bufs=4, space="PSUM") as ps:
        if True:
            wd_sb = big.tile([DH, FT, D], BF16)
            nc.sync.dma_start(out=wd_sb, in_=wdT[:].rearrange("(ft p) m -> p ft m", p=DH))
            h_sb = big.tile([DH, FT, TOK], BF16)
            sxm_sb = big.tile([DH, FT, TOK], BF16)
            g2_sb = big.tile([DH, FT, TOK], BF16)
            hg_sb = big.tile([DH, FT, TOK], BF16)
            for half in range(2):
                tsl = slice(half * HK, (half + 1) * HK)
                nc.sync.dma_start(
                    out=h_sb[:, :, tsl],
                    in_=h_i[:, tsl].rearrange("(ft p) t -> p ft t", p=DH))
                nc.sync.dma_start(
                    out=sxm_sb[:, :, tsl],
                    in_=sxm_i[:, tsl].rearrange("(ft p) t -> p ft t", p=DH))
                nc.sync.dma_start(
                    out=g2_sb[:, :, tsl],
                    in_=g2_i[:, tsl].rearrange("(ft p) t -> p ft t", p=DH))
                for ft in range(FT):
                    hs = ev.tile([DH, HK], BF16, tag="hs")
                    nc.vector.tensor_tensor(
                        hs[:], h_sb[:, ft, tsl], sxm_sb[:, ft, tsl], OP.add)
                    nc.vector.tensor_tensor(
                        hg_sb[:, ft, tsl], hs[:], g2_sb[:, ft, tsl], OP.mult)
                for m in range(MT):
                    for ns in range(2):
                        n0 = half * HK + ns * 512
                        pt = ps.tile([DH, 512], F32)
                        for kt in range(FT):
                            nc.tensor.matmul(
                                pt[:], wd_sb[:, kt, m * DH:(m + 1) * DH],
                                hg_sb[:, kt, n0:n0 + 512],
                                start=(kt == 0), stop=(kt == FT - 1))
                        ot = ev.tile([DH, 512], BF16, tag="ot")
                        if m % 2 == 0:
                            nc.scalar.copy(ot[:], pt[:])
                        else:
                            nc.vector.tensor_copy(ot[:], pt[:])
                        nc.sync.dma_start(
                            out=out_o[m * DH:(m + 1) * DH, n0:n0 + 512], in_=ot[:])
    nc.compile()
    return nc


# ------------------------------------------------------------- host glue ----
def host_gate_math(i_pre, f_pre):
    """i_pre, f_pre: [B, NH, S] f32.  Returns dict of f32 arrays.

    Exports (per b, h):
      esc   [S]        exp(a_j - m_new(chunk))            per-token k scale
      cch   [NCH]      exp(m_new - ms - scaG)             per-chunk Sp scale
      dec   [NCH]      exp(scaG + ms - m_new), col0 = 0   scan multiplier
      e2,e3 [NCH, LC]  den clamp terms (qk_scale folded)
    """
    i_pre = i_pre.astype(np.float64)
    f_pre = f_pre.astype(np.float64)
    vecI = np.log(1.0 / (1.0 + np.exp(-i_pre)) + EPS)
    vecF = np.log(1.0 / (1.0 + np.exp(-f_pre)) + EPS)
    Ic = vecI.reshape(B, NH, NCH, LC)
    Fc = vecF.reshape(B, NH, NCH, LC)
    vecB = np.cumsum(Fc, axis=-1)
    scaG = vecB[..., -1]
    vecA = scaG[..., None] - vecB + Ic

    ms = np.zeros((B, NH, NCH))
    dec = np.zeros((B, NH, NCH))
    m_new_arr = np.zeros((B, NH, NCH))
    m = np.zeros((B, NH))
    for c in range(NCH):
        amax = vecA[:, :, c, :].max(-1)
        m_new = np.maximum(scaG[:, :, c] + m, amax)
        ms[:, :, c] = m
        dec[:, :, c] = np.exp(scaG[:, :, c] + m - m_new)
        m_new_arr[:, :, c] = m_new
        m = m_new
    escale = np.exp(vecA - m_new_arr[..., None])          # [B,NH,NCH,LC]
    cch = np.exp(m_new_arr - ms - scaG)                   # [B,NH,NCH]

    mask = np.tril(np.ones((LC, LC), bool))
    logD = vecB[..., :, None] - vecB[..., None, :] + Ic[..., None, :]
    logD = np.where(mask, logD, -np.inf)
    m_intra = logD.max(-1)
    m_comb = np.maximum(vecB + ms[..., None], m_intra)
    e2 = np.exp(-vecB - ms[..., None]) / QK_SCALE
    e3 = EPS * np.exp(m_comb - vecB - ms[..., None]) / QK_SCALE
    dec0 = dec.copy()
    dec0[:, :, 0] = 0.0
    return dict(
        esc=escale.reshape(B, NH, S).astype(np.float32),
        cch=cch.astype(np.float32),
        dec=dec0.astype(np.float32),
        e2=e2.astype(np.float32), e3=e3.astype(np.float32))


def prep_weights(W_up, Wq, Wk, Wv, W_ig, W_fg, conv_w, conv_b, skip, W_down):
    """Host-side weight packing (same for all cores)."""
    FT = INNER // DH
    wupT = _bf(W_up.T)                                         # [512, 2048]
    wqkvT = _bf(np.concatenate([Wq.T, Wk.T, Wv.T], axis=1))    # [1024, 3072]
    wgT = _bf(np.concatenate([W_ig.T, W_fg.T], axis=1))        # [1024, 16]
    convw = np.ascontiguousarray(
        conv_w.reshape(FT, DH, KCONV).transpose(1, 0, 2).reshape(DH, FT * KCONV)
    ).astype(np.float32)
    convb = np.ascontiguousarray(conv_b.reshape(FT, DH).T).astype(np.float32)
    skip_p = np.ascontiguousarray(skip.reshape(FT, DH).T).astype(np.float32)
    wdT = _bf(W_down.T)                                        # [1024, 512]
    mask4 = _bf(np.tile(np.tril(np.ones((DH, DH), np.float32)), (1, 4)))
    return dict(wupT=wupT, wqkvT=wqkvT, wgT=wgT, convw=convw, convb=convb,
                skip_p=skip_p, wdT=wdT, mask4=mask4)


def build_a_inmaps(x, wp):
    """Per-core phase A input maps.  Core c = (b=c//4, quarter=c%4)."""
    in_maps = []
    for c in range(8):
        b, qt = c // 4, c % 4
        s0 = qt * TOK
        xs = x[b, :, :].T                                       # [512, S] view
        if s0 == 0:
            xt = np.concatenate([np.zeros((D, KCONV - 1), np.float32),
                                 xs[:, :TOK]], axis=1)
        else:
            xt = xs[:, s0 - (KCONV - 1): s0 + TOK]
        in_maps.append(dict(
            xt=_bf(xt), wupT=wp['wupT'], wqkvT=wp['wqkvT'], wgT=wp['wgT'],
            convw=wp['convw'], convb=wp['convb'], skip_i=wp['skip_p']))
    return in_maps


def assemble_a_outputs(a_results, b_ig, b_fg):
    """Concatenate per-core phase A outputs into full feature-major tensors."""
    def cat(name):
        return np.stack([
            np.concatenate([a_results[b * 4 + qt][name] for qt in range(4)], axis=1)
            for b in range(B)])
    q_t, k_t, v_t = cat('q_o'), cat('k_o'), cat('v_o')          # [B, INNER, S] bf16
    sxm_t, g2_t = cat('sxm_o'), cat('g2_o')
    gates = cat('gates_o').astype(np.float32)                   # [B, 16, S]
    i_pre = gates[:, :NH, :] + np.asarray(b_ig, np.float32)[None, :, None]
    f_pre = gates[:, NH:, :] + np.asarray(b_fg, np.float32)[None, :, None]
    return q_t, k_t, v_t, sxm_t, g2_t, i_pre, f_pre


def build_b_inmaps(q_t, k_t, v_t, g, wp):
    """Per-core phase B inputs.  Core c handles units (b, 2h) where
    b = c // 4, heads (2*(c%4), 2*(c%4)+1)."""
    in_maps = []
    for c in range(8):
        b, hp = c // 4, c % 4
        m = {'mask_i': wp['mask4']}
        for u in range(NUNIT):
            h = 2 * hp + u
            rs = slice(h * DH, (h + 1) * DH)
            kf = k_t[b, rs, :].astype(np.float32)               # [128, S] feat-major
            esc = g['esc'][b, h]                                # [S]
            cch_tok = np.repeat(g['cch'][b, h], LC)             # [S]
            m[f"qT{u}"] = np.ascontiguousarray(q_t[b, rs, :])
            m[f"kTc{u}"] = _bf(kf * (esc * cch_tok)[None, :])
            # token-major [p, c, d] / [p, c, e]
            kesc = (kf.T * esc[:, None]).reshape(NCH, LC, DH)
            m[f"kesc{u}"] = _bf(kesc.transpose(1, 0, 2).reshape(DH, NCH * DH))
            vone = np.empty((NCH, LC, NW), np.float32)
            vone[:, :, :DH] = v_t[b, rs, :].astype(np.float32).T.reshape(NCH, LC, DH)
            vone[:, :, DH] = 1.0
            m[f"vone{u}"] = _bf(vone.transpose(1, 0, 2).reshape(DH, NCH * NW))
            m[f"dec{u}"] = np.ascontiguousarray(
                np.broadcast_to(g['dec'][b, h][None, :], (DH, NCH)).astype(np.float32))
            m[f"e2{u}"] = np.ascontiguousarray(g['e2'][b, h].T.astype(np.float32))
            m[f"e3{u}"] = np.ascontiguousarray(g['e3'][b, h].T.astype(np.float32))
        in_maps.append(m)
    return in_maps


def build_c_inmaps(b_results, sxm_t, g2_t, wp):
    """Assemble h from phase B [p, c, e] layouts into feature-major h_t,
    then per-core phase C inputs."""
    h_t = np.empty((B, INNER, S), dtype=sxm_t.dtype)
    for c in range(8):
        b, hp = c // 4, c % 4
        for u in range(NUNIT):
            h = 2 * hp + u
            # [p, c, e] -> [e, c*LC + p]
            harr = np.asarray(b_results[c][f"h{u}"]).reshape(DH, NCH, DH)
            h_t[b, h * DH:(h + 1) * DH, :] = (
                harr.transpose(2, 1, 0).reshape(DH, S))
    in_maps = []
    for c in range(8):
        b, qt = c // 4, c % 4
        ts = slice(qt * TOK, (qt + 1) * TOK)
        in_maps.append(dict(
            h_i=np.ascontiguousarray(h_t[b, :, ts]),
            sxm_i=np.ascontiguousarray(sxm_t[b, :, ts]),
            g2_i=np.ascontiguousarray(g2_t[b, :, ts]),
            wdT=wp['wdT']))
    return in_maps


def assemble_output(c_results):
    out = np.empty((B, S, D), np.float32)
    for c in range(8):
        b, qt = c // 4, c % 4
        out[b, qt * TOK:(qt + 1) * TOK, :] = \
            np.asarray(c_results[c]['out_o']).astype(np.float32).T
    return out


# ------------------------------------------------------------------ entry ----
from concourse.bass_utils import run_bass_kernel_spmd as _run_spmd

_CACHE = {}


def _programs():
    if 'a' not in _CACHE:
        _CACHE['a'] = build_phase_a()
        _CACHE['b'] = build_phase_b()
        _CACHE['c'] = build_phase_c()
    return _CACHE['a'], _CACHE['b'], _CACHE['c']


def kernel(x, W_up, Wq, Wk, Wv, W_ig, b_ig, W_fg, b_fg, conv_w, conv_b, skip,
           W_down):
    x = np.asarray(x, np.float32)
    nc_a, nc_b, nc_c = _programs()
    cores = list(range(8))
    wp = prep_weights(W_up, Wq, Wk, Wv, W_ig, W_fg, conv_w, conv_b, skip, W_down)
    a_maps = build_a_inmaps(x, wp)
    ra = _run_spmd(nc_a, a_maps, core_ids=cores).results
    q_t, k_t, v_t, sxm_t, g2_t, i_pre, f_pre = assemble_a_outputs(ra, b_ig, b_fg)
    g = host_gate_math(i_pre, f_pre)
    b_maps = build_b_inmaps(q_t, k_t, v_t, g, wp)
    rb = _run_spmd(nc_b, b_maps, core_ids=cores).results
    c_maps = build_c_inmaps(rb, sxm_t, g2_t, wp)
    rc = _run_spmd(nc_c, c_maps, core_ids=cores).results
    return assemble_output(rc)
